# revision 13
# baseline (speedup 1.0000x reference)
"""LCNN conv2d kernel for Trainium2 (8 NeuronCores, batch-sharded).

Math: out[b,o,h,w] = sum_d Wmat[o,d] * conv2d(x, dictionary)[b,d,h,w]
where Wmat is the scatter-add of lookup_coefficients into [O, D].

Device strategy (per core, 2 batches):
 - stage 1: conv with the D=100 dictionary as 6 accumulating matmuls per
   output tile (kernel-width pairs packed into the 128-partition
   contraction via a +1-shifted copy of x on partitions 64..127).
 - stage 2: [O=256, D=100] channel-mix matmul on the conv result.
 - all matmul inputs rounded to float32r (full PE rate, ~1e-4 rel err).
"""
import os
import sys

for _p in ("/opt/trn_rl_repo", "/root/.axon_site/_ro/trn_rl_repo"):
    if os.path.isdir(_p) and _p not in sys.path:
        sys.path.insert(0, _p)

import ml_dtypes
import numpy as np
from contextlib import ExitStack

from concourse import bacc, mybir, tile
from concourse.bass_utils import run_bass_kernel_spmd

# problem shapes (hardcoded per contract)
B, CIN, H, W = 16, 64, 96, 96
D, O = 100, 256
NCORES = 8
BPC = B // NCORES          # batches per core
PH, PW = H + 2, W + 2      # zero-padded spatial
F = BPC * PH * PW          # per-partition x extent
R = 4                      # output rows per matmul tile
NT = H // R                # h-tiles per batch
G = 4                      # h-tiles per output-DMA group
NG = NT // G
N = R * W                  # matmul free size (384)
f32 = mybir.dt.float32
f32r = mybir.dt.float32r

_NC_CACHE = {}


def _build():
    nc = bacc.Bacc(None, target_bir_lowering=False, debug=False)
    # inputs are pre-rounded to f32r on the host so loads can use the fast
    # no-cast HWDGE path (SWDGE cast DMAs measured ~120 GB/s).
    xp = nc.declare_dram_parameter("xp", [CIN, F], f32r, isOutput=False)
    wp = nc.declare_dram_parameter("wp", [128, 3 * D], f32r, isOutput=False)
    ws = nc.declare_dram_parameter("ws", [128, 3 * D], f32r, isOutput=False)
    wm = nc.declare_dram_parameter("wm", [D, O], f32r, isOutput=False)
    out = nc.declare_dram_parameter("out", [BPC, O, H, W], f32, isOutput=True)

    with tile.TileContext(nc) as tc, ExitStack() as ctx:
        sb = ctx.enter_context(tc.tile_pool(name="sb", bufs=1))
        conv1p = ctx.enter_context(tc.tile_pool(name="conv1p", bufs=3))
        stgp = ctx.enter_context(tc.tile_pool(name="stgp", bufs=2))
        pcp = ctx.enter_context(tc.tile_pool(name="pcp", bufs=2, space="PSUM"))
        pop = ctx.enter_context(tc.tile_pool(name="pop", bufs=2, space="PSUM"))

        XX = sb.tile([128, F], f32r)
        wp_s = sb.tile([128, 3 * D], f32r)
        ws_s = sb.tile([128, 3 * D], f32r)
        wm_s = sb.tile([D, O], f32r)
        nc.sync.dma_start(wp_s[:], wp[:])
        nc.sync.dma_start(ws_s[:], ws[:])
        nc.sync.dma_start(wm_s[:], wm[:])

        # x load (f32 -> f32r cast in DMA) + the +1-shifted duplicate for
        # packing two kernel-width taps into one 128-deep contraction.
        NCH = 8
        L = F // NCH
        for c in range(NCH):
            a = c * L
            nc.sync.dma_start(XX[0:CIN, a:a + L], xp[:, a:a + L])
        for c in range(NCH):
            a = c * L
            e = min(a + L, F - 1)
            nc.vector.tensor_copy(XX[64:128, a:e], XX[0:CIN, a + 1:e + 1])
        # keep the one never-paired trailing element finite: the K=128-padded
        # single-tap matmuls read it under a zero weight (NaN would poison).
        nc.vector.tensor_copy(XX[64:128, F - 1:F], XX[0:CIN, F - 1:F])

        XXv = XX.rearrange("p (b h w) -> p b h w", b=BPC, h=PH, w=PW)

        for b in range(BPC):
            for g in range(NG):
                stg = stgp.tile([128, 2 * G * N], f32, name="stg")
                for t in range(G):
                    h0 = (g * G + t) * R
                    pc = pcp.tile([D, N], f32, name="pc")
                    for kh in range(3):
                        nc.tensor.matmul(
                            pc[:], wp_s[:, kh * D:(kh + 1) * D],
                            XXv[:, b, h0 + kh:h0 + kh + R, 0:W],
                            start=(kh == 0), stop=False)
                    for kh in range(3):
                        # K padded to 128 (zero weight rows 64..127) so the
                        # accumulation group has uniform K — mixed-K groups
                        # measured ~1.5-2.5x slower per matmul.
                        nc.tensor.matmul(
                            pc[:], ws_s[:, kh * D:(kh + 1) * D],
                            XXv[:, b, h0 + kh:h0 + kh + R, 2:PW],
                            start=False, stop=(kh == 2))
                    c1 = conv1p.tile([D, N], f32r, name="c1")
                    nc.vector.tensor_copy(c1[:], pc[:])
                    po0 = pop.tile([128, N], f32, name="po0")
                    po1 = pop.tile([128, N], f32, name="po1")
                    nc.tensor.matmul(po0[:], wm_s[:, 0:128], c1[:],
                                     start=True, stop=True)
                    nc.tensor.matmul(po1[:], wm_s[:, 128:256], c1[:],
                                     start=True, stop=True)
                    nc.scalar.copy(stg[:, t * N:(t + 1) * N], po0[:])
                    nc.scalar.copy(stg[:, G * N + t * N:G * N + (t + 1) * N],
                                   po1[:])
                    if t % 2 == 1:
                        # store half the group as soon as its two tiles are
                        # evacuated — shortens the kernel tail.
                        half = t // 2
                        dst = out[b].rearrange("(u o) h w -> o u (h w)", u=2)[
                            :, :,
                            g * G * N + half * 2 * N:
                            g * G * N + (half + 1) * 2 * N]
                        src = stg.rearrange("p (u n) -> p u n", u=2)[
                            :, :, half * 2 * N:(half + 1) * 2 * N]
                        nc.gpsimd.dma_start(dst, src)

    nc.compile()
    return nc


def _get_nc():
    if "nc" not in _NC_CACHE:
        _NC_CACHE["nc"] = _build()
    return _NC_CACHE["nc"]


def _round_f32r(a):
    # round to a bf16-pair representable value (what the fp32r datapath keeps)
    hi = a.astype(ml_dtypes.bfloat16).astype(np.float32)
    lo = (a - hi).astype(ml_dtypes.bfloat16).astype(np.float32)
    return hi + lo


def _prep_inputs(x, dictionary, lookup_coefficients, lookup_indices):
    x = np.asarray(x, dtype=np.float32)
    dic = np.asarray(dictionary, dtype=np.float32)
    coeff = np.asarray(lookup_coefficients, dtype=np.float32).reshape(O, -1)
    idx = np.asarray(lookup_indices).astype(np.int64).reshape(O, -1)

    wmat = np.zeros((O, D), np.float32)
    np.add.at(wmat, (np.arange(O)[:, None], idx), coeff)
    wm = np.ascontiguousarray(wmat.T)                     # [D, O]

    dt_ = dic.transpose(1, 0, 2, 3)                       # [cin, d, kh, kw]
    wp = np.zeros((128, 3 * D), np.float32)
    wsn = np.zeros((128, 3 * D), np.float32)              # rows 64.. stay zero
    for kh in range(3):
        wp[0:64, kh * D:(kh + 1) * D] = dt_[:, :, kh, 0]
        wp[64:128, kh * D:(kh + 1) * D] = dt_[:, :, kh, 1]
        wsn[0:64, kh * D:(kh + 1) * D] = dt_[:, :, kh, 2]

    xpad = np.zeros((B, CIN, PH, PW), np.float32)
    xpad[:, :, 1:H + 1, 1:W + 1] = _round_f32r(x)
    wp = _round_f32r(wp)
    wsn = _round_f32r(wsn)
    wm = _round_f32r(wm)

    in_maps = []
    for c in range(NCORES):
        xc = xpad[c * BPC:(c + 1) * BPC].transpose(1, 0, 2, 3).reshape(CIN, F)
        in_maps.append({
            "xp": np.ascontiguousarray(xc),
            "wp": wp, "ws": wsn, "wm": wm,
        })
    return in_maps


def _run(in_maps, trace=False, **kw):
    nc = _get_nc()
    return run_bass_kernel_spmd(nc, in_maps, core_ids=list(range(NCORES)),
                                trace=trace, **kw)


def kernel(x, dictionary, lookup_coefficients, lookup_indices):
    in_maps = _prep_inputs(x, dictionary, lookup_coefficients, lookup_indices)
    res = _run(in_maps)
    outs = [res.results[c]["out"] for c in range(NCORES)]
    return np.concatenate(outs, axis=0)
